# revision 23
# baseline (speedup 1.0000x reference)
"""AdditiveAttention (Bahdanau) distributed Bass kernel for 8 TRN2 NeuronCores.

Reference computation (per batch b):
    qp = queries[b] @ W_q                  # [Q, H]
    kp = keys[b]    @ W_k                  # [K, H]
    S[q,k]  = sum_h w_v[h] * tanh(qp[q,h] + kp[k,h])
    S masked to -1e6 for k >= valid_lens[b]
    attn = softmax(S, axis=k)
    out[b] = attn @ values[b]              # [Q, DV]

Key idea: tanh is replaced by a short sine series fit under the Gaussian
weight of x = qp+kp ~ N(0, 2):

    tanh(x) ~= sum_r a_r sin(w_r x)

Each sine term FACTORIZES across q and k:

    sin(w_r (qp+kp)) = sin(w_r qp) cos(w_r kp) + cos(w_r qp) sin(w_r kp)

so the [Q,K,H] pointwise tanh pass (the ACT-engine bottleneck of the
direct approach) collapses into 2R rank-H matmuls on the PE array:

    S^T[k,q] = sum_r sum_h [ sin_r(kp)[h,k] * (a_r w_v cos_r(qp))[h,q]
                           + cos_r(kp)[h,k] * (a_r w_v sin_r(qp))[h,q] ]

The ACT Sin spline is only accurate on |arg| <= ~4.2, and |proj| reaches
~5, so only the base frequencies {w0, 2w0, 3w0} are evaluated directly
(args <= ~5.1; out-of-domain hits are ~1-per-16k-tile and tiny).  The
higher frequencies {4w0, 6w0, 8w0} are derived on DVE with exact
double-angle identities (no large-argument sin ever evaluated):

    sin 2a = 2 sin a cos a,   cos 2a = 1 - 2 sin^2 a

Sin lives in the trig_and_small ACT table set, Exp in exp_and_others:
ALL sin calls are emitted before ALL exp calls so the ~2.7us table
switch happens once.

Per core: 2 full batches (16/8), each 4 k-blocks of 128 + 1 q-unit.
Engineering notes (each was measured to matter):
 - Projections run with the WEIGHTS as the stationary operand, so one
   FD=512 matmul projects all four k-blocks; the 5 units of a batch
   land in one [128, 5*128] PSUM tile.
 - ACT featurizes STRAIGHT from that PSUM tile using the activation
   unit's free affine: sin(scale*x) with scale=w_r (6 calls per batch,
   FD=640, freq-major order) — no scaled-copy matmuls, no staging hop.
   The high-frequency derive runs in two DVE stages (freq 4 after the
   freq-2 calls, freqs 6,8 after the freq-3 calls).
 - All bf16 inputs are host-packed into ONE [128, blob] array laid out
   exactly as SBUF wants it and moved by ~9 dma_starts in NEED order
   (weights+batch-0 featurize inputs split across queues first, values
   last).  Naive per-tensor rearranged DMAs produced ~4700 tiny
   descriptors and a ~14us ramp; single huge DMAs serialized on one
   queue.  Outputs are packed the same way ([NB, 128, 257] f32).
 - The k >= valid_len mask is added INTO the S^T PSUM chain by a rank-1
   matmul (mask row x ones row), so Exp needs no per-partition bias and
   runs as ONE call per batch over [128, 4*128]; the softmax denominator
   comes from four FD=1 matmuls against a ones column.
 - S^T chains are emitted derived-blocks-first; chain results are
   parked in SBUF f32 so all PE chain work overlaps the featurize
   phase.  PE reads of DMA-landed tiles (mask row, values) are fronted
   by a tiny ldweights touch so each matmul picks up at most one new
   semaphore.

exp needs no max-subtraction: |S| <= sum_r |a_r| * ||w_v||_1 ~ 15, and
masked positions get -29952 added -> exp == 0 exactly.  Host divides
numer by denom in f64.
"""

import math
import os

import numpy as np

import concourse.bacc as bacc
import concourse.bass as bass
import concourse.tile as tile
from concourse import mybir
from concourse.bass_utils import run_bass_kernel_spmd

B, Q, K, QS, KS, H, DV = 16, 128, 512, 256, 256, 128, 256
N_CORES = 8
NB = B // N_CORES  # batches per core
NT = K // 128      # k blocks per batch
NU = NT + 1        # units per batch (k blocks 0..3, q = 4)
MASK_NEG = -30000.0  # bf16-rounds to -29952; exp(S-29952) == 0 for |S|<=16

# Base frequencies evaluated by ACT Sin (f32 scale immediates); effective
# basis {w1, w2, w3, 2*w2, 2*w3, 4*w2} after DVE double-angle derivation.
# COEF fit against that basis under N(0, 1.5^2) weight on [-10,10].
WBASE = [0.36, 0.72]
COEF = [1.028222429117507, 0.28225502982973816, 0.18280491613315608,
        0.03583515474849887, 0.023314898012105706]
NF = 5            # total frequency blocks: freqs {1,2,4,6,8} * w0
NBASE = 2         # ACT-evaluated base blocks (freqs w0, 2*w0)
FW = NF * 128     # feature width per unit (768)

ND = QS // 128    # 128-row blocks in the projection contraction dim

# ---- input blob column layout (bf16, [128, BLOB_W]), need-ordered ----
O_WQ = 0                       # W_q blocks        [ND*H = 256]
O_WK = O_WQ + ND * H           # W_k blocks        [256]
O_QT0 = O_WK + ND * H          # queries^T b0      [ND*Q = 256]
O_KT0 = O_QT0 + ND * Q         # keys^T b0         [ND*K = 1024]
O_QT1 = O_KT0 + ND * K         # queries^T b1      [256]
O_KT1 = O_QT1 + ND * Q         # keys^T b1         [1024]
O_MR = O_KT1 + ND * K          # mask rows (row 0) [NB*NT*128 = 1024]
O_WVA = O_MR + NB * NT * 128   # a_r*w_v pattern   [768]
O_VV = O_WVA + FW              # values            [NB][NT*DV = 1024]
BLOB_W = O_VV + NB * NT * DV
O_QT = [O_QT0, O_QT1]
O_KT = [O_KT0, O_KT1]

F32 = mybir.dt.float32
BF16 = mybir.dt.bfloat16
MULT = mybir.AluOpType.mult
ADD = mybir.AluOpType.add
SIN = mybir.ActivationFunctionType.Sin
EXP = mybir.ActivationFunctionType.Exp

_BUILD_CACHE: dict = {}
LAST_RESULT = None  # BassKernelResults of the most recent run (for timing)


def _build() -> bass.Bass:
    nc = bacc.Bacc()

    blob = nc.declare_dram_parameter("blob", [128, BLOB_W], BF16, isOutput=False)
    ob = nc.declare_dram_parameter("ob", [NB, 128, DV + 1], F32, isOutput=True)

    with tile.TileContext(nc) as tc:
        with (
            tc.tile_pool(name="consts", bufs=1) as consts,
            tc.tile_pool(name="io", bufs=1) as io,
            tc.tile_pool(name="feat", bufs=1) as feat,
            tc.tile_pool(name="scr", bufs=2) as scr,
            tc.tile_pool(name="mid", bufs=2) as mid,
            tc.tile_pool(name="pproj", bufs=2, space="PSUM") as pproj,
            tc.tile_pool(name="psto", bufs=4, space="PSUM") as psto,
        ):
            # ---- input blob: dma_starts split for queue parallelism,
            # featurize-critical columns first ----
            bs = consts.tile([128, BLOB_W], BF16)
            cuts = [O_WQ, O_QT1, O_MR, BLOB_W]
            for a, b in zip(cuts[:-1], cuts[1:]):
                nc.sync.dma_start(out=bs[:, a:b], in_=blob[:, a:b])

            # ---- constants ----
            one_r = consts.tile([128, Q], BF16)  # row 0 = ones row
            nc.vector.memset(one_r[0:1, :], 1.0)
            one_c = consts.tile([128, 1], BF16)  # ones column (denominator)
            nc.vector.memset(one_c, 1.0)
            pih = consts.tile([128, 1], F32)  # pi/2 bias column for cos
            nc.vector.memset(pih, math.pi / 2)
            scl = consts.tile([128, NBASE], F32)  # per-freq ACT scales
            for r in range(NBASE):
                nc.vector.memset(scl[:, r : r + 1], WBASE[r])
            wu = consts.tile([128, 1], BF16)  # warmup: first biased Sin is
            nc.scalar.activation(out=wu, in_=pih, func=SIN, scale=scl[:, 0:1],
                                 bias=pih)  # ~1.3us one-time cost, hidden here
            wqk_b = io.tile([128, 2 * ND * H], BF16)  # staged: PE stationary
            nc.vector.tensor_copy(wqk_b, bs[:, O_WQ : O_WQ + 2 * ND * H])
            wva_b = io.tile([128, FW], BF16)  # staged: DVE fold operand
            nc.vector.tensor_copy(wva_b, bs[:, O_WVA : O_WVA + FW])

            def wq_blk(n):
                return wqk_b[:, n * H : (n + 1) * H]

            def wk_blk(n):
                return wqk_b[:, (ND + n) * H : (ND + n + 1) * H]

            # ---- featurize: proj (PE) -> ACT Sin with scale, per freq --
            kS = [None] * NB
            kC = [None] * NB
            pj = [None] * NB
            # touch: absorb the DVE semaphore for the staged weights so
            # projection matmuls only pick up the blob DMA semaphore.
            nc.tensor.ldweights(weights=wqk_b[:, 0:128])
            for j in range(NB):
                pj_ps = pproj.tile([128, NU, H], F32, tag="proj")
                for n in range(ND):
                    nc.tensor.matmul(
                        pj_ps[:, :NT, :], lhsT=wk_blk(n),
                        rhs=bs[:, O_KT[j] + n * K : O_KT[j] + (n + 1) * K],
                        start=(n == 0), stop=(n == ND - 1),
                    )
                for n in range(ND):
                    nc.tensor.matmul(
                        pj_ps[:, NT, :], lhsT=wq_blk(n),
                        rhs=bs[:, O_QT[j] + n * Q : O_QT[j] + (n + 1) * Q],
                        start=(n == 0), stop=(n == ND - 1),
                    )
                pj[j] = pj_ps
                # feature tiles: [H, unit, freq, 128]
                fs = feat.tile([H, NU, FW], BF16, tag=f"fs{j}")
                fc = feat.tile([H, NU, FW], BF16, tag=f"fc{j}")
                for r in (1, 0):  # freq-2 first: derives start early
                    nc.scalar.activation(
                        out=fs[:, :, r * 128 : (r + 1) * 128], in_=pj_ps,
                        func=SIN, scale=scl[:, r : r + 1],
                    )
                    nc.scalar.activation(
                        out=fc[:, :, r * 128 : (r + 1) * 128], in_=pj_ps,
                        func=SIN, scale=scl[:, r : r + 1], bias=pih,
                    )
                kS[j], kC[j] = fs, fc

            # ---- derive high blocks (DVE, 2 stages) + q-fold + chains --
            qsf = [None] * NB
            qcf = [None] * NB
            eb = [None] * NB
            # block order by readiness: freq-2 first, derived blocks,
            # freq-1 (block 0, featurized last) at the end.
            border = [1, 2, 3, 4, 0]
            al = slice(None)
            for j in range(NB):
                fs, fc = kS[j], kC[j]
                # blocks: 0=f1, 1=f2 (ACT); derived: 2=f4=2*f2,
                # 3=f6=3*f2 (triple angle), 4=f8=2*f4
                b1, b2 = slice(128, 256), slice(256, 384)
                b3, b4 = slice(384, 512), slice(512, 640)
                w2 = scr.tile([128, NU, 128], BF16, tag=f"w2{j}")
                nc.vector.tensor_tensor(w2, fs[al, al, b1], fs[al, al, b1], op=MULT)
                nc.vector.tensor_scalar(fc[al, al, b2], w2, -2.0, 1.0, op0=MULT, op1=ADD)
                nc.vector.scalar_tensor_tensor(
                    fs[al, al, b2], fs[al, al, b1], 2.0, fc[al, al, b1],
                    op0=MULT, op1=MULT,
                )
                v3 = scr.tile([128, NU, 128], BF16, tag=f"v3{j}")
                nc.vector.tensor_scalar(v3, w2, -4.0, 3.0, op0=MULT, op1=ADD)
                nc.vector.tensor_tensor(fs[al, al, b3], v3, fs[al, al, b1], op=MULT)
                nc.vector.scalar_tensor_tensor(
                    fc[al, al, b3], v3, -2.0, fc[al, al, b1],
                    op0=ADD, op1=MULT,
                )
                w4 = scr.tile([128, NU, 128], BF16, tag=f"w4{j}")
                nc.vector.tensor_tensor(w4, fs[al, al, b2], fs[al, al, b2], op=MULT)
                nc.vector.tensor_scalar(fc[al, al, b4], w4, -2.0, 1.0, op0=MULT, op1=ADD)
                nc.vector.scalar_tensor_tensor(
                    fs[al, al, b4], fs[al, al, b2], 2.0, fc[al, al, b2],
                    op0=MULT, op1=MULT,
                )
                # fold a_r * w_v[h] into the q features (unit 4), in two
                # readiness-ordered pieces: blocks 1..5 (freqs 2,3,4,6,8)
                # are done before the freq-1 calls; block 0 follows them.
                qsf_t = feat.tile([H, FW], BF16, tag=f"qsf{j}")
                nc.vector.tensor_tensor(
                    qsf_t[:, 128:], fs[:, NT, 128:], wva_b[:, 128:], op=MULT
                )
                qcf_t = feat.tile([H, FW], BF16, tag=f"qcf{j}")
                nc.vector.tensor_tensor(
                    qcf_t[:, 128:], fc[:, NT, 128:], wva_b[:, 128:], op=MULT
                )
                nc.vector.tensor_tensor(
                    qsf_t[:, :128], fs[:, NT, :128], wva_b[:, :128], op=MULT
                )
                nc.vector.tensor_tensor(
                    qcf_t[:, :128], fc[:, NT, :128], wva_b[:, :128], op=MULT
                )
                qsf[j], qcf[j] = qsf_t, qcf_t

            # ---- S^T chains; Exp straight from PSUM per k-block ----
            for j in range(NB):
                ej = mid.tile([128, NT, Q], BF16, tag=f"e{j}", bufs=1)
                for t in range(NT):
                    st_ps = psto.tile([128, Q], F32, tag="sto")
                    mi = O_MR + (j * NT + t) * 128
                    nc.tensor.matmul(
                        st_ps, lhsT=bs[0:1, mi : mi + 128], rhs=one_r[0:1, :],
                        start=True, stop=False,
                    )
                    for bi, rb in enumerate(border):
                        rsl = slice(rb * 128, (rb + 1) * 128)
                        nc.tensor.matmul(
                            st_ps, lhsT=kS[j][:, t, rsl], rhs=qcf[j][:, rsl],
                            start=False, stop=False,
                        )
                        nc.tensor.matmul(
                            st_ps, lhsT=kC[j][:, t, rsl], rhs=qsf[j][:, rsl],
                            start=False, stop=(bi == len(border) - 1),
                        )
                    nc.scalar.activation(out=ej[:, t, :], in_=st_ps, func=EXP)
                eb[j] = ej

            # ---- output (PE reads values straight from the blob) ----
            nc.tensor.ldweights(weights=bs[:, O_VV : O_VV + 128])
            for j in range(NB):
                o_ps = psto.tile([Q, DV + 1], F32, tag="sto")
                vo = O_VV + j * NT * DV
                for t in range(NT):
                    nc.tensor.matmul(
                        o_ps[:, :DV], lhsT=eb[j][:, t, :],
                        rhs=bs[:, vo + t * DV : vo + (t + 1) * DV],
                        start=(t == 0), stop=(t == NT - 1),
                    )
                for t in range(NT):
                    nc.tensor.matmul(
                        o_ps[:, DV : DV + 1], lhsT=eb[j][:, t, :], rhs=one_c,
                        start=(t == 0), stop=(t == NT - 1),
                    )
                o_sb = mid.tile([Q, DV + 1], F32, tag="osb")
                nc.vector.tensor_copy(o_sb, o_ps)
                nc.sync.dma_start(out=ob[j], in_=o_sb)

    nc.finalize()
    return nc


def kernel(queries, keys, values, valid_lens, W_q, W_k, w_v):
    import ml_dtypes

    queries = np.asarray(queries, dtype=np.float32)
    keys = np.asarray(keys, dtype=np.float32)
    values = np.asarray(values, dtype=np.float32)
    W_q = np.asarray(W_q, dtype=np.float32)
    W_k = np.asarray(W_k, dtype=np.float32)
    w_v = np.asarray(w_v, dtype=np.float32)
    vl = np.asarray(valid_lens).astype(np.int64)

    nc = _BUILD_CACHE.get("v6")
    if nc is None:
        nc = _build()
        _BUILD_CACHE["v6"] = nc

    bf = ml_dtypes.bfloat16
    wva_np = np.zeros((128, FW), np.float32)
    for r in range(NF):
        wva_np[:, r * 128 : (r + 1) * 128] = np.float32(COEF[r]) * w_v[:, None]

    kidx = np.arange(128)
    in_maps = []
    for c in range(N_CORES):
        bl = np.zeros((128, BLOB_W), bf)
        bl[:, O_WVA : O_WVA + FW] = wva_np
        for n in range(ND):
            bl[:, O_WQ + n * H : O_WQ + (n + 1) * H] = W_q[n * 128 : (n + 1) * 128]
            bl[:, O_WK + n * H : O_WK + (n + 1) * H] = W_k[n * 128 : (n + 1) * 128]
        for j in range(NB):
            b = c * NB + j
            qt = queries[b].T  # [QS, Q]
            kt = keys[b].T     # [KS, K]
            for n in range(ND):
                o = O_QT[j] + n * Q
                bl[:, o : o + Q] = qt[n * 128 : (n + 1) * 128]
                o = O_KT[j] + n * K
                bl[:, o : o + K] = kt[n * 128 : (n + 1) * 128]
            for t in range(NT):
                o = O_MR + (j * NT + t) * 128
                bl[0, o : o + 128] = np.where(
                    t * 128 + kidx < vl[b], 0.0, MASK_NEG
                ).astype(bf)
            o = O_VV + j * NT * DV
            bl[:, o : o + NT * DV] = np.ascontiguousarray(
                values[b].reshape(NT, 128, DV).transpose(1, 0, 2).reshape(128, NT * DV)
            )
        in_maps.append({"blob": bl})

    global LAST_RESULT
    res = run_bass_kernel_spmd(
        nc,
        in_maps,
        core_ids=list(range(N_CORES)),
        trace=bool(os.environ.get("KERNEL_TRACE")),
    )
    LAST_RESULT = res

    out = np.zeros((B, Q, DV), np.float32)
    for c in range(N_CORES):
        obc = res.results[c]["ob"].astype(np.float64)  # [NB, 128, DV+1]
        for j in range(NB):
            out[c * NB + j] = obc[j, :, :DV] / obc[j, :, DV][:, None]
    return out.astype(np.float32)


# revision 24
# speedup vs baseline: 1.0443x; 1.0443x over previous
"""AdditiveAttention (Bahdanau) distributed Bass kernel for 8 TRN2 NeuronCores.

Reference computation (per batch b):
    qp = queries[b] @ W_q                  # [Q, H]
    kp = keys[b]    @ W_k                  # [K, H]
    S[q,k]  = sum_h w_v[h] * tanh(qp[q,h] + kp[k,h])
    S masked to -1e6 for k >= valid_lens[b]
    attn = softmax(S, axis=k)
    out[b] = attn @ values[b]              # [Q, DV]

Key idea: tanh is replaced by a short sine series fit under the Gaussian
weight of x = qp+kp ~ N(0, 2):

    tanh(x) ~= sum_r a_r sin(w_r x)

Each sine term FACTORIZES across q and k:

    sin(w_r (qp+kp)) = sin(w_r qp) cos(w_r kp) + cos(w_r qp) sin(w_r kp)

so the [Q,K,H] pointwise tanh pass (the ACT-engine bottleneck of the
direct approach) collapses into 2R rank-H matmuls on the PE array:

    S^T[k,q] = sum_r sum_h [ sin_r(kp)[h,k] * (a_r w_v cos_r(qp))[h,q]
                           + cos_r(kp)[h,k] * (a_r w_v sin_r(qp))[h,q] ]

The ACT Sin spline is only accurate on |arg| <= ~4.2, and |proj| reaches
~5, so only the base frequencies {w0, 2w0, 3w0} are evaluated directly
(args <= ~5.1; out-of-domain hits are ~1-per-16k-tile and tiny).  The
higher frequencies {4w0, 6w0, 8w0} are derived on DVE with exact
double-angle identities (no large-argument sin ever evaluated):

    sin 2a = 2 sin a cos a,   cos 2a = 1 - 2 sin^2 a

Sin lives in the trig_and_small ACT table set, Exp in exp_and_others:
ALL sin calls are emitted before ALL exp calls so the ~2.7us table
switch happens once.

Per core: 2 full batches (16/8), each 4 k-blocks of 128 + 1 q-unit.
Engineering notes (each was measured to matter):
 - Projections run with the WEIGHTS as the stationary operand, so one
   FD=512 matmul projects all four k-blocks; the 5 units of a batch
   land in one [128, 5*128] PSUM tile.
 - ACT featurizes STRAIGHT from that PSUM tile using the activation
   unit's free affine: sin(scale*x) with scale=w_r (6 calls per batch,
   FD=640, freq-major order) — no scaled-copy matmuls, no staging hop.
   The high-frequency derive runs in two DVE stages (freq 4 after the
   freq-2 calls, freqs 6,8 after the freq-3 calls).
 - All bf16 inputs are host-packed into ONE [128, blob] array laid out
   exactly as SBUF wants it and moved by ~9 dma_starts in NEED order
   (weights+batch-0 featurize inputs split across queues first, values
   last).  Naive per-tensor rearranged DMAs produced ~4700 tiny
   descriptors and a ~14us ramp; single huge DMAs serialized on one
   queue.  Outputs are packed the same way ([NB, 128, 257] f32).
 - The k >= valid_len mask is added INTO the S^T PSUM chain by a rank-1
   matmul (mask row x ones row), so Exp needs no per-partition bias and
   runs as ONE call per batch over [128, 4*128]; the softmax denominator
   comes from four FD=1 matmuls against a ones column.
 - S^T chains are emitted derived-blocks-first; chain results are
   parked in SBUF f32 so all PE chain work overlaps the featurize
   phase.  PE reads of DMA-landed tiles (mask row, values) are fronted
   by a tiny ldweights touch so each matmul picks up at most one new
   semaphore.

exp needs no max-subtraction: |S| <= sum_r |a_r| * ||w_v||_1 ~ 15, and
masked positions get -29952 added -> exp == 0 exactly.  Host divides
numer by denom in f64.
"""

import math
import os

import numpy as np

import concourse.bacc as bacc
import concourse.bass as bass
import concourse.tile as tile
from concourse import mybir
from concourse.bass_utils import run_bass_kernel_spmd

B, Q, K, QS, KS, H, DV = 16, 128, 512, 256, 256, 128, 256
N_CORES = 8
NB = B // N_CORES  # batches per core
NT = K // 128      # k blocks per batch
NU = NT + 1        # units per batch (k blocks 0..3, q = 4)
MASK_NEG = -30000.0  # bf16-rounds to -29952; exp(S-29952) == 0 for |S|<=16

# Base frequencies evaluated by ACT Sin (f32 scale immediates); effective
# basis {w1, w2, w3, 2*w2, 2*w3, 4*w2} after DVE double-angle derivation.
# COEF fit against that basis under N(0, 1.5^2) weight on [-10,10].
WBASE = [0.36, 0.72]
COEF = [1.028222429117507, 0.28225502982973816, 0.18280491613315608,
        0.03583515474849887, 0.023314898012105706]
NF = 5            # total frequency blocks: freqs {1,2,4,6,8} * w0
NBASE = 2         # ACT-evaluated base blocks (freqs w0, 2*w0)
FW = NF * 128     # feature width per unit (768)

ND = QS // 128    # 128-row blocks in the projection contraction dim

# ---- input blob column layout (bf16, [128, BLOB_W]), need-ordered ----
O_WQ = 0                       # W_q blocks        [ND*H = 256]
O_WK = O_WQ + ND * H           # W_k blocks        [256]
O_QT0 = O_WK + ND * H          # queries^T b0      [ND*Q = 256]
O_KT0 = O_QT0 + ND * Q         # keys^T b0         [ND*K = 1024]
O_QT1 = O_KT0 + ND * K         # queries^T b1      [256]
O_KT1 = O_QT1 + ND * Q         # keys^T b1         [1024]
O_MR = O_KT1 + ND * K          # mask rows (row 0) [NB*NT*128 = 1024]
O_WVA = O_MR + NB * NT * 128   # a_r*w_v pattern   [768]
O_VV = O_WVA + FW              # values            [NB][NT*DV = 1024]
BLOB_W = O_VV + NB * NT * DV
O_QT = [O_QT0, O_QT1]
O_KT = [O_KT0, O_KT1]

F32 = mybir.dt.float32
BF16 = mybir.dt.bfloat16
MULT = mybir.AluOpType.mult
ADD = mybir.AluOpType.add
SIN = mybir.ActivationFunctionType.Sin
EXP = mybir.ActivationFunctionType.Exp

_BUILD_CACHE: dict = {}
LAST_RESULT = None  # BassKernelResults of the most recent run (for timing)


def _build() -> bass.Bass:
    nc = bacc.Bacc()

    blob = nc.declare_dram_parameter("blob", [128, BLOB_W], BF16, isOutput=False)
    ob = nc.declare_dram_parameter("ob", [NB, 128, DV + 1], F32, isOutput=True)

    with tile.TileContext(nc) as tc:
        with (
            tc.tile_pool(name="consts", bufs=1) as consts,
            tc.tile_pool(name="io", bufs=1) as io,
            tc.tile_pool(name="feat", bufs=1) as feat,
            tc.tile_pool(name="scr", bufs=2) as scr,
            tc.tile_pool(name="mid", bufs=2) as mid,
            tc.tile_pool(name="pproj", bufs=2, space="PSUM") as pproj,
            tc.tile_pool(name="psto", bufs=4, space="PSUM") as psto,
        ):
            # ---- input blob: dma_starts split for queue parallelism,
            # featurize-critical columns first ----
            bs = consts.tile([128, BLOB_W], BF16)
            cuts = [O_WQ, O_QT1, O_MR, BLOB_W]
            for a, b in zip(cuts[:-1], cuts[1:]):
                nc.sync.dma_start(out=bs[:, a:b], in_=blob[:, a:b])

            # ---- constants ----
            one_r = consts.tile([128, Q], BF16)  # row 0 = ones row
            nc.vector.memset(one_r[0:1, :], 1.0)
            one_c = consts.tile([128, 1], BF16)  # ones column (denominator)
            nc.vector.memset(one_c, 1.0)
            pih = consts.tile([128, 1], F32)  # pi/2 bias column for cos
            nc.vector.memset(pih, math.pi / 2)
            scl = consts.tile([128, NBASE], F32)  # per-freq ACT scales
            for r in range(NBASE):
                nc.vector.memset(scl[:, r : r + 1], WBASE[r])
            wu = consts.tile([128, 1], BF16)  # warmup: first biased Sin is
            nc.scalar.activation(out=wu, in_=pih, func=SIN, scale=scl[:, 0:1],
                                 bias=pih)  # ~1.3us one-time cost, hidden here
            wqk_b = io.tile([128, 2 * ND * H], BF16)  # staged: PE stationary
            nc.vector.tensor_copy(wqk_b, bs[:, O_WQ : O_WQ + 2 * ND * H])
            wva_b = io.tile([128, FW], BF16)  # staged: DVE fold operand
            nc.vector.tensor_copy(wva_b, bs[:, O_WVA : O_WVA + FW])

            def wq_blk(n):
                return wqk_b[:, n * H : (n + 1) * H]

            def wk_blk(n):
                return wqk_b[:, (ND + n) * H : (ND + n + 1) * H]

            # ---- featurize: proj (PE) -> ACT Sin with scale, per freq --
            kS = [None] * NB
            kC = [None] * NB
            pj = [None] * NB
            # touch: absorb the DVE semaphore for the staged weights so
            # projection matmuls only pick up the blob DMA semaphore.
            nc.tensor.ldweights(weights=wqk_b[:, 0:128])
            for j in range(NB):
                pj_ps = pproj.tile([128, NU, H], F32, tag="proj")
                for n in range(ND):
                    nc.tensor.matmul(
                        pj_ps[:, :NT, :], lhsT=wk_blk(n),
                        rhs=bs[:, O_KT[j] + n * K : O_KT[j] + (n + 1) * K],
                        start=(n == 0), stop=(n == ND - 1),
                    )
                for n in range(ND):
                    nc.tensor.matmul(
                        pj_ps[:, NT, :], lhsT=wq_blk(n),
                        rhs=bs[:, O_QT[j] + n * Q : O_QT[j] + (n + 1) * Q],
                        start=(n == 0), stop=(n == ND - 1),
                    )
                pj[j] = pj_ps
                # feature tiles: [H, unit, freq, 128]
                fs = feat.tile([H, NU, FW], BF16, tag=f"fs{j}")
                fc = feat.tile([H, NU, FW], BF16, tag=f"fc{j}")
                for r in (1, 0):  # freq-2 first: derives start early
                    nc.scalar.activation(
                        out=fs[:, :, r * 128 : (r + 1) * 128], in_=pj_ps,
                        func=SIN, scale=scl[:, r : r + 1],
                    )
                    nc.scalar.activation(
                        out=fc[:, :, r * 128 : (r + 1) * 128], in_=pj_ps,
                        func=SIN, scale=scl[:, r : r + 1], bias=pih,
                    )
                kS[j], kC[j] = fs, fc

            # ---- derive high blocks (DVE, 2 stages) + q-fold + chains --
            qsf = [None] * NB
            qcf = [None] * NB
            eb = [None] * NB
            # block order by fold/feature readiness: freq-2 (call 2),
            # freq-1 (call 4), then the derived blocks.
            border = [1, 0, 2, 3, 4]
            al = slice(None)
            for j in range(NB):
                fs, fc = kS[j], kC[j]
                # q-fold of block 1 (freq 2) only needs the 2nd ACT call:
                # emit it first so chains can start during featurize.
                qsf_t = feat.tile([H, FW], BF16, tag=f"qsf{j}")
                qcf_t = feat.tile([H, FW], BF16, tag=f"qcf{j}")
                s12 = slice(128, 256)
                nc.vector.tensor_tensor(
                    qsf_t[:, s12], fs[:, NT, s12], wva_b[:, s12], op=MULT
                )
                nc.vector.tensor_tensor(
                    qcf_t[:, s12], fc[:, NT, s12], wva_b[:, s12], op=MULT
                )
                # blocks: 0=f1, 1=f2 (ACT); derived: 2=f4=2*f2,
                # 3=f6=3*f2 (triple angle), 4=f8=2*f4
                b1, b2 = slice(128, 256), slice(256, 384)
                b3, b4 = slice(384, 512), slice(512, 640)
                w2 = scr.tile([128, NU, 128], BF16, tag=f"w2{j}")
                nc.vector.tensor_tensor(w2, fs[al, al, b1], fs[al, al, b1], op=MULT)
                nc.vector.tensor_scalar(fc[al, al, b2], w2, -2.0, 1.0, op0=MULT, op1=ADD)
                nc.vector.scalar_tensor_tensor(
                    fs[al, al, b2], fs[al, al, b1], 2.0, fc[al, al, b1],
                    op0=MULT, op1=MULT,
                )
                v3 = scr.tile([128, NU, 128], BF16, tag=f"v3{j}")
                nc.vector.tensor_scalar(v3, w2, -4.0, 3.0, op0=MULT, op1=ADD)
                nc.vector.tensor_tensor(fs[al, al, b3], v3, fs[al, al, b1], op=MULT)
                nc.vector.scalar_tensor_tensor(
                    fc[al, al, b3], v3, -2.0, fc[al, al, b1],
                    op0=ADD, op1=MULT,
                )
                w4 = scr.tile([128, NU, 128], BF16, tag=f"w4{j}")
                nc.vector.tensor_tensor(w4, fs[al, al, b2], fs[al, al, b2], op=MULT)
                nc.vector.tensor_scalar(fc[al, al, b4], w4, -2.0, 1.0, op0=MULT, op1=ADD)
                nc.vector.scalar_tensor_tensor(
                    fs[al, al, b4], fs[al, al, b2], 2.0, fc[al, al, b2],
                    op0=MULT, op1=MULT,
                )
                # fold a_r * w_v[h] into the q features (unit 4), in two
                # readiness-ordered pieces: blocks 1..5 (freqs 2,3,4,6,8)
                # are done before the freq-1 calls; block 0 follows them.
                # remaining folds by readiness: block 0 (after call 4),
                # then the derived blocks 2..4.
                nc.vector.tensor_tensor(
                    qsf_t[:, :128], fs[:, NT, :128], wva_b[:, :128], op=MULT
                )
                nc.vector.tensor_tensor(
                    qcf_t[:, :128], fc[:, NT, :128], wva_b[:, :128], op=MULT
                )
                nc.vector.tensor_tensor(
                    qsf_t[:, 256:], fs[:, NT, 256:], wva_b[:, 256:], op=MULT
                )
                nc.vector.tensor_tensor(
                    qcf_t[:, 256:], fc[:, NT, 256:], wva_b[:, 256:], op=MULT
                )
                qsf[j], qcf[j] = qsf_t, qcf_t

            # ---- S^T chains; Exp straight from PSUM per k-block ----
            for j in range(NB):
                ej = mid.tile([128, NT, Q], BF16, tag=f"e{j}", bufs=1)
                for t in range(NT):
                    st_ps = psto.tile([128, Q], F32, tag="sto")
                    mi = O_MR + (j * NT + t) * 128
                    nc.tensor.matmul(
                        st_ps, lhsT=bs[0:1, mi : mi + 128], rhs=one_r[0:1, :],
                        start=True, stop=False,
                    )
                    for bi, rb in enumerate(border):
                        rsl = slice(rb * 128, (rb + 1) * 128)
                        nc.tensor.matmul(
                            st_ps, lhsT=kS[j][:, t, rsl], rhs=qcf[j][:, rsl],
                            start=False, stop=False,
                        )
                        nc.tensor.matmul(
                            st_ps, lhsT=kC[j][:, t, rsl], rhs=qsf[j][:, rsl],
                            start=False, stop=(bi == len(border) - 1),
                        )
                    nc.scalar.activation(out=ej[:, t, :], in_=st_ps, func=EXP)
                eb[j] = ej

            # ---- output (PE reads values straight from the blob) ----
            nc.tensor.ldweights(weights=bs[:, O_VV : O_VV + 128])
            for j in range(NB):
                o_ps = psto.tile([Q, DV + 1], F32, tag="sto")
                vo = O_VV + j * NT * DV
                for t in range(NT):
                    nc.tensor.matmul(
                        o_ps[:, :DV], lhsT=eb[j][:, t, :],
                        rhs=bs[:, vo + t * DV : vo + (t + 1) * DV],
                        start=(t == 0), stop=(t == NT - 1),
                    )
                for t in range(NT):
                    nc.tensor.matmul(
                        o_ps[:, DV : DV + 1], lhsT=eb[j][:, t, :], rhs=one_c,
                        start=(t == 0), stop=(t == NT - 1),
                    )
                o_sb = mid.tile([Q, DV + 1], F32, tag="osb")
                nc.vector.tensor_copy(o_sb, o_ps)
                nc.sync.dma_start(out=ob[j], in_=o_sb)

    nc.finalize()
    return nc


def kernel(queries, keys, values, valid_lens, W_q, W_k, w_v):
    import ml_dtypes

    queries = np.asarray(queries, dtype=np.float32)
    keys = np.asarray(keys, dtype=np.float32)
    values = np.asarray(values, dtype=np.float32)
    W_q = np.asarray(W_q, dtype=np.float32)
    W_k = np.asarray(W_k, dtype=np.float32)
    w_v = np.asarray(w_v, dtype=np.float32)
    vl = np.asarray(valid_lens).astype(np.int64)

    nc = _BUILD_CACHE.get("v6")
    if nc is None:
        nc = _build()
        _BUILD_CACHE["v6"] = nc

    bf = ml_dtypes.bfloat16
    wva_np = np.zeros((128, FW), np.float32)
    for r in range(NF):
        wva_np[:, r * 128 : (r + 1) * 128] = np.float32(COEF[r]) * w_v[:, None]

    kidx = np.arange(128)
    in_maps = []
    for c in range(N_CORES):
        bl = np.zeros((128, BLOB_W), bf)
        bl[:, O_WVA : O_WVA + FW] = wva_np
        for n in range(ND):
            bl[:, O_WQ + n * H : O_WQ + (n + 1) * H] = W_q[n * 128 : (n + 1) * 128]
            bl[:, O_WK + n * H : O_WK + (n + 1) * H] = W_k[n * 128 : (n + 1) * 128]
        for j in range(NB):
            b = c * NB + j
            qt = queries[b].T  # [QS, Q]
            kt = keys[b].T     # [KS, K]
            for n in range(ND):
                o = O_QT[j] + n * Q
                bl[:, o : o + Q] = qt[n * 128 : (n + 1) * 128]
                o = O_KT[j] + n * K
                bl[:, o : o + K] = kt[n * 128 : (n + 1) * 128]
            for t in range(NT):
                o = O_MR + (j * NT + t) * 128
                bl[0, o : o + 128] = np.where(
                    t * 128 + kidx < vl[b], 0.0, MASK_NEG
                ).astype(bf)
            o = O_VV + j * NT * DV
            bl[:, o : o + NT * DV] = np.ascontiguousarray(
                values[b].reshape(NT, 128, DV).transpose(1, 0, 2).reshape(128, NT * DV)
            )
        in_maps.append({"blob": bl})

    global LAST_RESULT
    res = run_bass_kernel_spmd(
        nc,
        in_maps,
        core_ids=list(range(N_CORES)),
        trace=bool(os.environ.get("KERNEL_TRACE")),
    )
    LAST_RESULT = res

    out = np.zeros((B, Q, DV), np.float32)
    for c in range(N_CORES):
        obc = res.results[c]["ob"].astype(np.float64)  # [NB, 128, DV+1]
        for j in range(NB):
            out[c * NB + j] = obc[j, :, :DV] / obc[j, :, DV][:, None]
    return out.astype(np.float32)
